# revision 13
# baseline (speedup 1.0000x reference)
"""MiniMaxText01 linear attention layer on 8 trn2 NeuronCores.

Tensor-parallel over heads (4 heads/core), single fused pass per core:
for each 512-token tile, project q/k/v/gate (weights SBUF-resident),
run blocked lightning attention (BLOCK=256, kv state in SBUF), gate,
and emit the partial out-projection. No DRAM staging of intermediates.

Precision: the q/k/v projections run in fp8-e4m3 with DoubleRow packing
(2 K-rows per PE cell -> ~1.8x matmul throughput); weights are scaled
by 64 on the host to clear the e4m3 subnormal range and the 1/64
dequant is folded into the silu activation scale.  The gate and output
projections and all attention matmuls stay bf16 with fp32 PSUM
accumulation (measured end-to-end rel err ~1.5e-2 vs the 2e-2 gate;
full-fp8 everywhere measures ~4.3e-2 and fails).  kv state is fp32
with a bf16 operand copy.  The gate sigmoid is computed as
0.5*(1+tanh(a/2)) so every ACT function (silu, tanh, copy) lives in
one activation-table set (no ~2.7us table swaps); the 0.5 and the
RMSNorm weight are folded into w_out on the host.

Pipeline shaping: weights load once (outside the repeat loop) and the
first x-tile DMA is emitted before them so the PE can start early; each
tile's out-projection is deferred until after the next tile's
projection MMs are emitted, so the gA join latency hides under them.

Per-core outputs: pout = (gate*attn*normw) @ w_out  [4096, 2048] bf16
and ssq = sum_c attn^2 [1, 4096]; host sums across cores and applies
rsqrt(var + eps).
"""
import math
import numpy as np
import ml_dtypes
from contextlib import ExitStack

import concourse.bass as bass
import concourse.tile as tile
import concourse.mybir as mybir
from concourse import bacc
from concourse.bass_utils import run_bass_kernel_spmd

FP32 = mybir.dt.float32
BF16 = mybir.dt.bfloat16
FP8 = mybir.dt.float8e4
AF = mybir.ActivationFunctionType
DR = mybir.MatmulPerfMode.DoubleRow

SEQ = 4096
HIDDEN = 2048
NUM_HEADS = 32
HEAD_DIM = 128
INNER = NUM_HEADS * HEAD_DIM
BLOCK = 256
EPS = 1e-5
N_CORES = 8
HPC = NUM_HEADS // N_CORES          # 4 heads per core
IN_PC = HPC * HEAD_DIM              # 512 inner channels per core
P = 128

CH = 512                            # token tile
NT = SEQ // CH                      # 8 tiles
HC = HIDDEN // P                    # 16 hidden chunks
HH = HC // 2

# fp8 chunk counts (of HC=16) per projection, from the low-K side.
# 0 = pure bf16, 16 = pure fp8 DoubleRow.  out-proj is always bf16.
N8_Q = 16
N8_K = 16
N8_V = 16
N8_G = 4
W8SCALE = 64.0                      # host-side weight scale for fp8 paths


def build_nc(repeat: int = 1, phases: str = "AB", nA: int = 1, nB: int = 1):
    nc = bacc.Bacc("TRN2", target_bir_lowering=False)

    any8 = max(N8_Q, N8_K, N8_V, N8_G) > 0

    xt_d = nc.dram_tensor("xt", [HIDDEN, SEQ], BF16, kind="ExternalInput")
    if any8:
        xt8_d = nc.dram_tensor("xt8", [HIDDEN, SEQ], FP8, kind="ExternalInput")

    def w_pair(name, n8):
        d8 = dbf = None
        if n8 > 0:
            d8 = nc.dram_tensor(name + "8", [n8 * P, IN_PC], FP8,
                                kind="ExternalInput")
        if n8 < HC:
            dbf = nc.dram_tensor(name, [(HC - n8) * P, IN_PC], BF16,
                                 kind="ExternalInput")
        return d8, dbf

    wq8_d, wq_d = w_pair("wq", N8_Q)
    wk8_d, wk_d = w_pair("wk", N8_K)
    wv8_d, wv_d = w_pair("wv", N8_V)
    wg8_d, wg_d = w_pair("wg", N8_G)
    wo_d = nc.dram_tensor("wo", [IN_PC, HIDDEN], BF16, kind="ExternalInput")
    qdec_d = nc.dram_tensor("qdec", [HPC, P, BLOCK], BF16, kind="ExternalInput")
    dmask_d = nc.dram_tensor("dmask", [HPC, P, BLOCK], BF16, kind="ExternalInput")
    kdec_d = nc.dram_tensor("kdec", [HPC, 2, P, 1], FP32, kind="ExternalInput")
    bdi_d = nc.dram_tensor("bdi", [HPC, P, P], BF16, kind="ExternalInput")
    ident_d = nc.dram_tensor("ident", [P, P], BF16, kind="ExternalInput")
    ones_d = nc.dram_tensor("ones", [P, 1], BF16, kind="ExternalInput")
    kv0_d = nc.dram_tensor("kv0", [HPC, P, P], FP32, kind="ExternalInput")

    pout_d = nc.dram_tensor("pout", [SEQ, HIDDEN], BF16, kind="ExternalOutput")
    ssq_d = nc.dram_tensor("ssq", [1, SEQ], FP32, kind="ExternalOutput")

    with tile.TileContext(nc) as tc, ExitStack() as ctx:
        const = ctx.enter_context(tc.tile_pool(name="const", bufs=1))
        psum = ctx.enter_context(tc.tile_pool(name="psum", bufs=1, space="PSUM"))
        sb = ctx.enter_context(tc.tile_pool(name="sb", bufs=1))

        xt_r = xt_d[:].rearrange("(hc p) n -> p hc n", p=P)
        if any8:
            xt8_r = xt8_d[:].rearrange("(hc p) n -> p hc n", p=P)

        # First tile's x (fp8 then bf16 halves) and the q weights ahead of
        # the other weights (shared FIFO rings execute in emission order)
        # so the first projection matmul can start as early as possible.
        xh8_pre = [None, None]
        xh_pre = [None, None]
        if any8:
            x8 = sb.tile([P, HH, CH], FP8, tag="xt8", bufs=4)
            nc.sync.dma_start(x8[:], xt8_r[:, 0:HH, 0:CH])
            xh8_pre[0] = x8

        def w_tiles(d8, dbf, n8, tag):
            # each weight DMA is split in chunk-halves so consumers of the
            # low-K chunks don't wait for the whole tensor
            t8 = tbf = None
            if d8 is not None:
                t8 = const.tile([P, n8, IN_PC], FP8, tag=tag + "8")
                r8 = d8[:].rearrange("(hc p) m -> p hc m", p=P)
                nc.sync.dma_start(t8[:, 0:n8 // 2, :], r8[:, 0:n8 // 2, :])
                nc.sync.dma_start(t8[:, n8 // 2:n8, :], r8[:, n8 // 2:n8, :])
            if dbf is not None:
                nbf = HC - n8
                tbf = const.tile([P, nbf, IN_PC], BF16, tag=tag)
                rbf = dbf[:].rearrange("(hc p) m -> p hc m", p=P)
                nc.sync.dma_start(tbf[:, 0:nbf // 2, :], rbf[:, 0:nbf // 2, :])
                nc.sync.dma_start(tbf[:, nbf // 2:nbf, :], rbf[:, nbf // 2:nbf, :])
            return t8, tbf

        wq8_t, wq_t = w_tiles(wq8_d, wq_d, N8_Q, "wq")
        if any8:
            x8 = sb.tile([P, HH, CH], FP8, tag="xt8", bufs=4)
            nc.sync.dma_start(x8[:], xt8_r[:, HH:HC, 0:CH])
            xh8_pre[1] = x8
        wk8_t, wk_t = w_tiles(wk8_d, wk_d, N8_K, "wk")
        wv8_t, wv_t = w_tiles(wv8_d, wv_d, N8_V, "wv")
        xh = sb.tile([P, HH, CH], BF16, tag="xt", bufs=4)
        nc.sync.dma_start(xh[:], xt_r[:, 0:HH, 0:CH])
        xh_pre[0] = xh
        xh = sb.tile([P, HH, CH], BF16, tag="xt", bufs=4)
        nc.sync.dma_start(xh[:], xt_r[:, HH:HC, 0:CH])
        xh_pre[1] = xh
        wg8_t, wg_t = w_tiles(wg8_d, wg_d, N8_G, "wg")
        wo_t = const.tile([P, HPC, HIDDEN], BF16)
        nc.sync.dma_start(wo_t[:], wo_d[:].rearrange("(h p) n -> p h n", p=P))

        ident_t = const.tile([P, P], BF16)
        nc.sync.dma_start(ident_t[:], ident_d[:])
        ones_t = const.tile([P, 1], BF16)
        nc.sync.dma_start(ones_t[:], ones_d[:])
        qdec_t = const.tile([P, HPC, BLOCK], BF16)
        nc.sync.dma_start(qdec_t[:], qdec_d[:].rearrange("h p i -> p h i"))
        dmask_t = const.tile([P, HPC, BLOCK], BF16)
        nc.sync.dma_start(dmask_t[:], dmask_d[:].rearrange("h p i -> p h i"))
        kdec_t = const.tile([P, HPC, 2, 1], FP32)
        nc.sync.dma_start(kdec_t[:], kdec_d[:].rearrange("h s p o -> p h s o"))
        bdi_t = const.tile([P, HPC, P], BF16)
        nc.sync.dma_start(bdi_t[:], bdi_d[:].rearrange("h d e -> d h e"))
        kv_t = const.tile([P, HPC, P], FP32)
        kvb_t = const.tile([P, HPC, P], BF16)

        pending_out = None      # (attn, g1, tile index) awaiting tail work;
        # carried across repeats so only the very last tile pays an
        # unoverlapped tail and repeat boundaries pipeline like tile ones
        for _rep in range(repeat):
            nc.sync.dma_start(kv_t[:], kv0_d[:].rearrange("h d e -> d h e"))
            nc.scalar.copy(kvb_t[:], kv_t[:])

            def emit_ssq(attn_t, t, c0=0, c1=CH):
                w = c1 - c0
                sq_t = sb.tile([P, 2, CH], BF16, tag="sq", bufs=2)
                nc.vector.tensor_mul(sq_t[:, 0, :w], attn_t[:, 0, c0:c1],
                                     attn_t[:, 0, c0:c1])
                for hh2 in range(1, HPC):
                    nc.vector.tensor_mul(sq_t[:, 1, :w], attn_t[:, hh2, c0:c1],
                                         attn_t[:, hh2, c0:c1])
                    nc.vector.tensor_add(sq_t[:, 0, :w],
                                         sq_t[:, 0, :w].bitcast(BF16),
                                         sq_t[:, 1, :w])
                pss = psum.tile([1, CH], FP32, tag="psA", bufs=2)
                nc.tensor.matmul(pss[:, :w], ones_t[:], sq_t[:, 0, :w],
                                 start=True, stop=True)
                ssb = sb.tile([1, CH], FP32, tag="ssb", bufs=2)
                nc.scalar.copy(ssb[:, :w], pss[:, :w])
                nc.sync.dma_start(ssq_d[:, t * CH + c0:t * CH + c1],
                                  ssb[:, :w])

            def emit_outproj(gA_t, t, ms):
                for m in ms:
                    ob = sb.tile([P, HIDDEN], BF16, tag="ob", bufs=2)
                    for nt in range(4):
                        pso2 = psum.tile([P, 512], FP32, tag="psA", bufs=2)
                        for h in range(HPC):
                            nc.tensor.matmul(
                                pso2[:],
                                gA_t[:, h, m * P:(m + 1) * P],
                                wo_t[:, h, nt * 512:(nt + 1) * 512],
                                start=(h == 0), stop=(h == HPC - 1))
                        nc.vector.tensor_copy(ob[:, nt * 512:(nt + 1) * 512],
                                              pso2[:])
                    nc.sync.dma_start(
                        pout_d[t * CH + m * P: t * CH + (m + 1) * P, :], ob[:])


            for t in range(NT):
                tsl = slice(t * CH, (t + 1) * CH)

                # ---- projections (streaming x tile, weights resident) ----
                if t == 0 and _rep == 0:
                    xt_h = xh_pre
                    xt8_h = xh8_pre
                else:
                    xt_h = []
                    xt8_h = []
                    for half in range(2):
                        if any8:
                            x8 = sb.tile([P, HH, CH], FP8, tag="xt8", bufs=4)
                            nc.sync.dma_start(
                                x8[:], xt8_r[:, half * HH:(half + 1) * HH, tsl])
                            xt8_h.append(x8)
                    for half in range(2):
                        xh = sb.tile([P, HH, CH], BF16, tag="xt", bufs=4)
                        nc.sync.dma_start(
                            xh[:], xt_r[:, half * HH:(half + 1) * HH, tsl])
                        xt_h.append(xh)

                q_t = sb.tile([P, HPC, CH], BF16, tag="q", bufs=2)
                k_t = sb.tile([P, HPC, CH], BF16, tag="k", bufs=2)
                g1_t = sb.tile([P, HPC, CH], BF16, tag="g1", bufs=2)
                v_t = sb.tile([P, 4, IN_PC], BF16, tag="v", bufs=2)

                def emit_proj(w8_t, w_t, n8, dst):
                    nmm_tot = n8 // 2 + (HC - n8)
                    descale = (1.0 / W8SCALE) if n8 > 0 else 1.0
                    for h in range(HPC):
                        hsl = slice(h * P, (h + 1) * P)
                        ps = psum.tile([P, CH], FP32, tag="psA", bufs=2)
                        nmm = 0
                        for i8 in range(0, n8, 2):
                            nc.tensor.matmul(
                                ps[:], w8_t[:, i8:i8 + 2, hsl],
                                xt8_h[i8 // HH][:, i8 % HH:i8 % HH + 2, :],
                                start=(nmm == 0), stop=(nmm == nmm_tot - 1),
                                perf_mode=DR)
                            nmm += 1
                        for hc in range(n8, HC):
                            nc.tensor.matmul(
                                ps[:], w_t[:, hc - n8, hsl],
                                xt_h[hc // HH][:, hc % HH, :],
                                start=(nmm == 0), stop=(nmm == nmm_tot - 1))
                            nmm += 1
                        if dst is g1_t:
                            # tanh(a/2); gate = (1+tanh(a/2))/2, 1/2 is in wo
                            nc.scalar.activation(g1_t[:, h, :], ps[:], AF.Tanh,
                                                 scale=0.5 * descale)
                            nc.vector.tensor_scalar_add(
                                g1_t[:, h, :], g1_t[:, h, :].bitcast(BF16), 1.0)
                        else:
                            nc.scalar.activation(dst[:, h, :], ps[:], AF.Silu,
                                                 scale=descale)

                emit_proj(wq8_t, wq_t, N8_Q, q_t)
                emit_proj(wk8_t, wk_t, N8_K, k_t)
                nmm_tot = N8_V // 2 + (HC - N8_V)
                descale = (1.0 / W8SCALE) if N8_V > 0 else 1.0
                for t2 in range(4):
                    ps = psum.tile([P, IN_PC], FP32, tag="psA", bufs=2)
                    nmm = 0
                    for i8 in range(0, N8_V, 2):
                        nc.tensor.matmul(
                            ps[:],
                            xt8_h[i8 // HH][:, i8 % HH:i8 % HH + 2,
                                            t2 * P:(t2 + 1) * P],
                            wv8_t[:, i8:i8 + 2, :],
                            start=(nmm == 0), stop=(nmm == nmm_tot - 1),
                            perf_mode=DR)
                        nmm += 1
                    for hc in range(N8_V, HC):
                        nc.tensor.matmul(
                            ps[:],
                            xt_h[hc // HH][:, hc % HH, t2 * P:(t2 + 1) * P],
                            wv_t[:, hc - N8_V, :],
                            start=(nmm == 0), stop=(nmm == nmm_tot - 1))
                        nmm += 1
                    nc.scalar.activation(v_t[:, t2, :], ps[:], AF.Silu,
                                         scale=descale)

                # gate proj is only consumed by the NEXT tile's gA multiply
                # (except on the very last tile), so emit it after the
                # attention blocks — its weights/x arrive later in the DMA
                # stream and the attention MMs give them runway.
                last = t == NT - 1 and _rep == repeat - 1
                if last:
                    emit_proj(wg8_t, wg_t, N8_G, g1_t)

                # ---- previous tile's gate + ssq (DVE chains get a full
                # tile of runway); its out-projection is interleaved into
                # the attention blocks below as PE filler ----
                prev = None
                if pending_out is not None:
                    pa, pg, pt = pending_out
                    gA_prev = sb.tile([P, HPC, CH], BF16, tag="gA", bufs=2)
                    nc.vector.tensor_mul(gA_prev[:], pa[:], pg[:])
                    emit_ssq(pa, pt)
                    prev = (gA_prev, pt)

                attn_t = sb.tile([P, HPC, CH], BF16, tag="attn", bufs=2)
                if last:
                    gA_t = sb.tile([P, HPC, CH], BF16, tag="gA", bufs=2)

                # ---- blocked lightning attention ----
                for b in range(CH // BLOCK):
                    t0 = b * BLOCK
                    qd = sb.tile([P, HPC, BLOCK], BF16, tag="qd", bufs=2)
                    nc.vector.tensor_mul(qd[:], q_t[:, :, t0:t0 + BLOCK],
                                         qdec_t[:])
                    s0s, s1s, kns = [], [], []
                    for h in range(HPC):
                        # scores (transposed): s[j, i] = k_j . q_i
                        ps0 = psum.tile([P, BLOCK], FP32, tag="ps_s", bufs=2)
                        nc.tensor.matmul(ps0[:], k_t[:, h, t0:t0 + P],
                                         q_t[:, h, t0:t0 + BLOCK],
                                         start=True, stop=True)
                        s0 = sb.tile([P, BLOCK], BF16, tag="s0", bufs=3)
                        nc.vector.tensor_mul(s0[:], ps0[:], dmask_t[:, h, :])
                        ps1 = psum.tile([P, P], FP32, tag="ps_s", bufs=2)
                        nc.tensor.matmul(ps1[:], k_t[:, h, t0 + P:t0 + BLOCK],
                                         q_t[:, h, t0 + P:t0 + BLOCK],
                                         start=True, stop=True)
                        s1 = sb.tile([P, P], BF16, tag="s1", bufs=3)
                        nc.vector.tensor_mul(s1[:], ps1[:], dmask_t[:, h, :P])
                        # k natural (for kv update), k-decay folded in
                        kn = []
                        for sub in range(2):
                            pst = psum.tile([P, P], BF16, tag="ps_tr", bufs=2)
                            nc.tensor.transpose(
                                pst[:],
                                k_t[:, h, t0 + sub * P:t0 + (sub + 1) * P],
                                ident_t[:])
                            knt = sb.tile([P, P], BF16, tag=f"kn{sub}", bufs=3)
                            nc.scalar.activation(knt[:], pst[:], AF.Copy,
                                                 scale=kdec_t[:, h, sub, :])
                            kn.append(knt)
                        s0s.append(s0); s1s.append(s1); kns.append(kn)
                    # out-proj chunk of tile t-1: PE filler while this
                    # block's masks (DVE) and kv chain land
                    if prev is not None:
                        emit_outproj(prev[0], prev[1], (2 * b, 2 * b + 1))
                    for h in range(HPC):
                        hsl = slice(h * P, (h + 1) * P)
                        # attention out (transposed): inter + intra
                        pso = psum.tile([P, BLOCK], FP32, tag="ps_o", bufs=2)
                        nc.tensor.matmul(pso[:], kvb_t[:, h, :], qd[:, h, :],
                                         start=True, stop=False)
                        nc.tensor.matmul(pso[:], v_t[:, 2 * b, hsl], s0s[h][:],
                                         start=False, stop=False)
                        nc.tensor.matmul(pso[:, P:BLOCK], v_t[:, 2 * b + 1, hsl],
                                         s1s[h][:], start=False, stop=True,
                                         skip_group_check=True)
                        nc.scalar.copy(attn_t[:, h, t0:t0 + BLOCK], pso[:])
                        # kv update: kv = (bd*I)^T kv + (k*kdec)^T v  (one group)
                        psk = psum.tile([P, P], FP32, tag="ps_s", bufs=2)
                        nc.tensor.matmul(psk[:], bdi_t[:, h, :], kvb_t[:, h, :],
                                         start=True, stop=False)
                        nc.tensor.matmul(psk[:], kns[h][0][:], v_t[:, 2 * b, hsl],
                                         start=False, stop=False)
                        nc.tensor.matmul(psk[:], kns[h][1][:],
                                         v_t[:, 2 * b + 1, hsl],
                                         start=False, stop=True)
                        nc.scalar.copy(kvb_t[:, h, :], psk[:])

                    if last:
                        # stream the final tile's tail per half-block so the
                        # kernel doesn't end with an unoverlapped join
                        csl = slice(t0, t0 + BLOCK)
                        nc.vector.tensor_mul(gA_t[:, :, csl],
                                             attn_t[:, :, csl],
                                             g1_t[:, :, csl])
                        emit_outproj(gA_t, t, (2 * b, 2 * b + 1))
                        emit_ssq(attn_t, t, t0, t0 + BLOCK)

                if not last:
                    emit_proj(wg8_t, wg_t, N8_G, g1_t)
                    pending_out = (attn_t, g1_t, t)

    nc.compile()
    return nc


_NC_CACHE = {}


def _get_nc(repeat=1, phases="AB", nA=1, nB=1):
    key = (repeat, phases, nA, nB)
    if key not in _NC_CACHE:
        _NC_CACHE[key] = build_nc(repeat, phases, nA, nB)
    return _NC_CACHE[key]


def make_in_maps(inputs):
    np_bf = ml_dtypes.bfloat16
    np_f8 = ml_dtypes.float8_e4m3

    hs = np.ascontiguousarray(np.asarray(inputs["hidden_states"], dtype=np.float32))
    w_qkv = np.asarray(inputs["w_qkv"], dtype=np.float32)
    w_gate = np.asarray(inputs["w_gate"], dtype=np.float32)
    w_out = np.asarray(inputs["w_out"], dtype=np.float32)
    norm_weight = np.asarray(inputs["norm_weight"], dtype=np.float32)
    slope_rate = np.asarray(inputs["slope_rate"], dtype=np.float32).reshape(NUM_HEADS)
    kv_cache = np.asarray(inputs["kv_cache"], dtype=np.float32)

    xt_f32 = np.ascontiguousarray(hs.T)                   # [HIDDEN, SEQ]
    xt = xt_f32.astype(np_bf)
    any8 = max(N8_Q, N8_K, N8_V, N8_G) > 0
    if any8:
        xt8 = np.clip(xt_f32, -240, 240).astype(np_f8)
    wq3 = w_qkv.reshape(HIDDEN, NUM_HEADS, 3 * HEAD_DIM)
    ident = np.eye(P, dtype=np_bf)
    ones = np.ones((P, 1), dtype=np_bf)
    idx = np.arange(BLOCK, dtype=np.float64)

    def w_split(m, w, n8):
        """fp8 low-K part (scaled) + bf16 remainder (scaled iff n8>0)."""
        out = {}
        if n8 > 0:
            out[m + "8"] = np.ascontiguousarray(
                np.clip(w[:n8 * P] * W8SCALE, -240, 240)).astype(np_f8)
            if n8 < HC:
                out[m] = np.ascontiguousarray(
                    w[n8 * P:] * W8SCALE).astype(np_bf)
        else:
            out[m] = np.ascontiguousarray(w).astype(np_bf)
        return out

    in_maps = []
    for c in range(N_CORES):
        s = slope_rate[c * HPC:(c + 1) * HPC].astype(np.float64)  # [HPC]
        wq = np.ascontiguousarray(
            wq3[:, c * HPC:(c + 1) * HPC, 0:HEAD_DIM].reshape(HIDDEN, IN_PC))
        wk = np.ascontiguousarray(
            wq3[:, c * HPC:(c + 1) * HPC, HEAD_DIM:2 * HEAD_DIM].reshape(HIDDEN, IN_PC))
        wv = np.ascontiguousarray(
            wq3[:, c * HPC:(c + 1) * HPC, 2 * HEAD_DIM:3 * HEAD_DIM].reshape(HIDDEN, IN_PC))
        wg = np.ascontiguousarray(w_gate[:, c * IN_PC:(c + 1) * IN_PC])
        # fold RMSNorm weight and the gate's 1/2 into w_out rows
        nw = norm_weight[c * IN_PC:(c + 1) * IN_PC].reshape(IN_PC, 1)
        wo = np.ascontiguousarray(
            0.5 * nw * w_out[c * IN_PC:(c + 1) * IN_PC, :]).astype(np_bf)

        jj = idx[:P][:, None]                            # [128,1]
        ii = idx[None, :]                                # [1,256]
        dmask = (np.exp(-s[:, None, None] * (ii - jj)) * (ii >= jj)).astype(np_bf)
        qdec = np.broadcast_to(
            np.exp(-s[:, None] * (idx[None, :] + 1.0))[:, None, :],
            (HPC, P, BLOCK)).astype(np_bf)
        kdec = np.exp(-s[:, None] * (BLOCK - 1.0 - idx[None, :]))  # [HPC, 256]
        kdec = kdec.reshape(HPC, 2, P, 1).astype(np.float32)
        bdi = (np.exp(-s * BLOCK)[:, None, None]
               * np.eye(P, dtype=np.float64)[None]).astype(np_bf)
        kv0 = np.ascontiguousarray(kv_cache[c * HPC:(c + 1) * HPC])

        im = {
            "xt": xt,
            "wo": wo,
            "qdec": np.ascontiguousarray(qdec),
            "dmask": np.ascontiguousarray(dmask),
            "kdec": kdec,
            "bdi": np.ascontiguousarray(bdi),
            "ident": ident, "ones": ones, "kv0": kv0,
        }
        if any8:
            im["xt8"] = xt8
        im.update(w_split("wq", wq, N8_Q))
        im.update(w_split("wk", wk, N8_K))
        im.update(w_split("wv", wv, N8_V))
        im.update(w_split("wg", wg, N8_G))
        in_maps.append(im)
    return in_maps


def combine_outputs(results):
    pout = results[0]["pout"].astype(np.float32)
    ssq = results[0]["ssq"].reshape(SEQ).astype(np.float32).copy()
    for r in results[1:]:
        pout += r["pout"].astype(np.float32)
        ssq += r["ssq"].reshape(SEQ)
    scale = 1.0 / np.sqrt(ssq / INNER + EPS)
    return (pout * scale[:, None]).astype(np.float32)


def kernel(**inputs):
    nc = _get_nc(1)
    in_maps = make_in_maps(inputs)
    res = run_bass_kernel_spmd(nc, in_maps, core_ids=list(range(N_CORES)))
    return combine_outputs(res.results)


# revision 22
# speedup vs baseline: 1.0947x; 1.0947x over previous
"""MiniMaxText01 linear attention layer on 8 trn2 NeuronCores.

Tensor-parallel over heads (4 heads/core), single fused pass per core:
for each 512-token tile, project q/k/v/gate (weights SBUF-resident),
run blocked lightning attention (BLOCK=256, kv state in SBUF), gate,
and emit the partial out-projection. No DRAM staging of intermediates.

Precision: the q/k/v projections run in fp8-e4m3 with DoubleRow packing
(2 K-rows per PE cell -> ~1.8x matmul throughput); weights are scaled
by 64 on the host to clear the e4m3 subnormal range and the 1/64
dequant is folded into the silu activation scale.  The gate and output
projections and all attention matmuls stay bf16 with fp32 PSUM
accumulation (measured end-to-end rel err ~1.5e-2 vs the 2e-2 gate;
full-fp8 everywhere measures ~4.3e-2 and fails).  kv state is fp32
with a bf16 operand copy.  The gate sigmoid is computed as
0.5*(1+tanh(a/2)) so every ACT function (silu, tanh, copy) lives in
one activation-table set (no ~2.7us table swaps); the 0.5 and the
RMSNorm weight are folded into w_out on the host.

Pipeline shaping: weights load once (outside the repeat loop) and the
first x-tile DMA is emitted before them so the PE can start early; each
tile's out-projection is deferred until after the next tile's
projection MMs are emitted, so the gA join latency hides under them.

Per-core outputs: pout = (gate*attn*normw) @ w_out  [4096, 2048] bf16
and ssq = sum_c attn^2 [1, 4096]; host sums across cores and applies
rsqrt(var + eps).
"""
import math
import numpy as np
import ml_dtypes
from contextlib import ExitStack

import concourse.bass as bass
import concourse.tile as tile
import concourse.mybir as mybir
from concourse import bacc
from concourse.bass_utils import run_bass_kernel_spmd

FP32 = mybir.dt.float32
BF16 = mybir.dt.bfloat16
FP8 = mybir.dt.float8e4
AF = mybir.ActivationFunctionType
DR = mybir.MatmulPerfMode.DoubleRow

SEQ = 4096
HIDDEN = 2048
NUM_HEADS = 32
HEAD_DIM = 128
INNER = NUM_HEADS * HEAD_DIM
BLOCK = 256
EPS = 1e-5
N_CORES = 8
HPC = NUM_HEADS // N_CORES          # 4 heads per core
IN_PC = HPC * HEAD_DIM              # 512 inner channels per core
P = 128

CH = 512                            # token tile
NT = SEQ // CH                      # 8 tiles
HC = HIDDEN // P                    # 16 hidden chunks
HH = HC // 2

# fp8 chunk counts (of HC=16) per projection, from the low-K side.
# 0 = pure bf16, 16 = pure fp8 DoubleRow.  out-proj is always bf16.
N8_Q = 16
N8_K = 16
N8_V = 16
# gate: NOS_G chunks run one-sided fp8 DoubleRow (weights exact via hi/lo
# e4m3 slot pairs, x carries the e4m3 error); the rest stay bf16
NOS_G = 12
W8SCALE = 64.0                      # host-side weight scale for fp8 paths


def build_nc(repeat: int = 1, phases: str = "AB", nA: int = 1, nB: int = 1):
    nc = bacc.Bacc("TRN2", target_bir_lowering=False)

    any8 = max(N8_Q, N8_K, N8_V, NOS_G) > 0

    xt_d = nc.dram_tensor("xt", [HIDDEN, SEQ], BF16, kind="ExternalInput")
    if any8:
        xt8_d = nc.dram_tensor("xt8", [HIDDEN, SEQ], FP8, kind="ExternalInput")
    if NOS_G > 0:
        # x8/16 (exponent-shifted copy): the lo-slot moving operand for the
        # one-sided gate matmuls
        xt8s_d = nc.dram_tensor("xt8s", [HIDDEN, SEQ], FP8, kind="ExternalInput")

    def w_pair(name, n8):
        d8 = dbf = None
        if n8 > 0:
            d8 = nc.dram_tensor(name + "8", [n8 * P, IN_PC], FP8,
                                kind="ExternalInput")
        if n8 < HC:
            dbf = nc.dram_tensor(name, [(HC - n8) * P, IN_PC], BF16,
                                 kind="ExternalInput")
        return d8, dbf

    wq8_d, wq_d = w_pair("wq", N8_Q)
    wk8_d, wk_d = w_pair("wk", N8_K)
    wv8_d, wv_d = w_pair("wv", N8_V)
    wg8_d = wg_d = None
    if NOS_G > 0:
        # hi/lo e4m3 slot pairs of the (x64-scaled) gate weights
        wg8_d = nc.dram_tensor("wg8", [NOS_G * P, 2, IN_PC], FP8,
                               kind="ExternalInput")
    if NOS_G < HC:
        wg_d = nc.dram_tensor("wg", [(HC - NOS_G) * P, IN_PC], BF16,
                              kind="ExternalInput")
    wo_d = nc.dram_tensor("wo", [IN_PC, HIDDEN], BF16, kind="ExternalInput")
    qdec_d = nc.dram_tensor("qdec", [HPC, P, BLOCK], BF16, kind="ExternalInput")
    dmask_d = nc.dram_tensor("dmask", [HPC, P, BLOCK], BF16, kind="ExternalInput")
    kdec_d = nc.dram_tensor("kdec", [HPC, 2, P, 1], FP32, kind="ExternalInput")
    bdi_d = nc.dram_tensor("bdi", [HPC, P, P], BF16, kind="ExternalInput")
    ident_d = nc.dram_tensor("ident", [P, P], BF16, kind="ExternalInput")
    ones_d = nc.dram_tensor("ones", [P, 1], BF16, kind="ExternalInput")
    kv0_d = nc.dram_tensor("kv0", [HPC, P, P], FP32, kind="ExternalInput")

    pout_d = nc.dram_tensor("pout", [SEQ, HIDDEN], BF16, kind="ExternalOutput")
    ssq_d = nc.dram_tensor("ssq", [1, SEQ], FP32, kind="ExternalOutput")

    with tile.TileContext(nc) as tc, ExitStack() as ctx:
        const = ctx.enter_context(tc.tile_pool(name="const", bufs=1))
        psum = ctx.enter_context(tc.tile_pool(name="psum", bufs=1, space="PSUM"))
        sb = ctx.enter_context(tc.tile_pool(name="sb", bufs=1))

        xt_r = xt_d[:].rearrange("(hc p) n -> p hc n", p=P)
        if any8:
            xt8_r = xt8_d[:].rearrange("(hc p) n -> p hc n", p=P)
        if NOS_G > 0:
            xt8s_r = xt8s_d[:].rearrange("(hc p) n -> p hc n", p=P)
        NV = 2 if NOS_G > 0 else 1      # x8 tile versions: [x8, x8/16]

        def x8_tile(half, tsl, vers=(0,)):
            """allocate the fp8 x tile for one chunk-half; DMA only `vers`
            planes (plane 1, the /16 copy, is only read by the gate)."""
            x8 = sb.tile([P, NV, HH, CH], FP8, tag="xt8", bufs=4)
            for v in vers:
                src = xt8_r if v == 0 else xt8s_r
                nc.sync.dma_start(x8[:, v, :, :],
                                  src[:, half * HH:(half + 1) * HH, tsl])
            return x8

        # First tile's x (fp8 then bf16 halves) and the q weights ahead of
        # the other weights (shared FIFO rings execute in emission order)
        # so the first projection matmul can start as early as possible.
        xh8_pre = [None, None]
        xh_pre = [None, None]
        if any8:
            xh8_pre[0] = x8_tile(0, slice(0, CH))

        def w_tiles(d8, dbf, n8, tag):
            # each weight DMA is split in chunk-halves so consumers of the
            # low-K chunks don't wait for the whole tensor
            t8 = tbf = None
            if d8 is not None:
                t8 = const.tile([P, n8, IN_PC], FP8, tag=tag + "8")
                r8 = d8[:].rearrange("(hc p) m -> p hc m", p=P)
                nc.sync.dma_start(t8[:, 0:n8 // 2, :], r8[:, 0:n8 // 2, :])
                nc.sync.dma_start(t8[:, n8 // 2:n8, :], r8[:, n8 // 2:n8, :])
            if dbf is not None:
                nbf = HC - n8
                tbf = const.tile([P, nbf, IN_PC], BF16, tag=tag)
                rbf = dbf[:].rearrange("(hc p) m -> p hc m", p=P)
                nc.sync.dma_start(tbf[:, 0:nbf // 2, :], rbf[:, 0:nbf // 2, :])
                nc.sync.dma_start(tbf[:, nbf // 2:nbf, :], rbf[:, nbf // 2:nbf, :])
            return t8, tbf

        wq8_t, wq_t = w_tiles(wq8_d, wq_d, N8_Q, "wq")
        if any8:
            xh8_pre[1] = x8_tile(1, slice(0, CH))
        wk8_t, wk_t = w_tiles(wk8_d, wk_d, N8_K, "wk")
        wv8_t, wv_t = w_tiles(wv8_d, wv_d, N8_V, "wv")
        if NOS_G > 0:
            for half in range(2):
                nc.sync.dma_start(
                    xh8_pre[half][:, 1, :, :],
                    xt8s_r[:, half * HH:(half + 1) * HH, 0:CH])
        xh = sb.tile([P, HH, CH], BF16, tag="xt", bufs=4)
        nc.sync.dma_start(xh[:], xt_r[:, 0:HH, 0:CH])
        xh_pre[0] = xh
        xh = sb.tile([P, HH, CH], BF16, tag="xt", bufs=4)
        nc.sync.dma_start(xh[:], xt_r[:, HH:HC, 0:CH])
        xh_pre[1] = xh
        wg8_t = wg_t = None
        if NOS_G > 0:
            wg8_t = const.tile([P, NOS_G, 2, IN_PC], FP8, tag="wg8")
            rg8 = wg8_d[:].rearrange("(hc p) two m -> p hc two m", p=P)
            nc.sync.dma_start(wg8_t[:, 0:NOS_G // 2], rg8[:, 0:NOS_G // 2])
            nc.sync.dma_start(wg8_t[:, NOS_G // 2:NOS_G], rg8[:, NOS_G // 2:NOS_G])
        if NOS_G < HC:
            ngbf = HC - NOS_G
            wg_t = const.tile([P, ngbf, IN_PC], BF16, tag="wg")
            rgbf = wg_d[:].rearrange("(hc p) m -> p hc m", p=P)
            nc.sync.dma_start(wg_t[:], rgbf[:])
        wo_t = const.tile([P, HPC, HIDDEN], BF16)
        nc.sync.dma_start(wo_t[:], wo_d[:].rearrange("(h p) n -> p h n", p=P))

        ident_t = const.tile([P, P], BF16)
        nc.sync.dma_start(ident_t[:], ident_d[:])
        ones_t = const.tile([P, 1], BF16)
        nc.sync.dma_start(ones_t[:], ones_d[:])
        qdec_t = const.tile([P, HPC, BLOCK], BF16)
        nc.sync.dma_start(qdec_t[:], qdec_d[:].rearrange("h p i -> p h i"))
        dmask_t = const.tile([P, HPC, BLOCK], BF16)
        nc.sync.dma_start(dmask_t[:], dmask_d[:].rearrange("h p i -> p h i"))
        kdec_t = const.tile([P, HPC, 2, 1], FP32)
        nc.sync.dma_start(kdec_t[:], kdec_d[:].rearrange("h s p o -> p h s o"))
        bdi_t = const.tile([P, HPC, P], BF16)
        nc.sync.dma_start(bdi_t[:], bdi_d[:].rearrange("h d e -> d h e"))
        kv_t = const.tile([P, HPC, P], FP32)
        kvb_t = const.tile([P, HPC, P], BF16)

        pending_out = None      # (attn, g1, tile index) awaiting tail work;
        # carried across repeats so only the very last tile pays an
        # unoverlapped tail and repeat boundaries pipeline like tile ones
        for _rep in range(repeat):
            nc.sync.dma_start(kv_t[:], kv0_d[:].rearrange("h d e -> d h e"))
            nc.scalar.copy(kvb_t[:], kv_t[:])

            def emit_ssq(attn_t, t, c0=0, c1=CH):
                w = c1 - c0
                sq_t = sb.tile([P, 2, CH], BF16, tag="sq", bufs=2)
                nc.vector.tensor_mul(sq_t[:, 0, :w], attn_t[:, 0, c0:c1],
                                     attn_t[:, 0, c0:c1])
                for hh2 in range(1, HPC):
                    nc.vector.tensor_mul(sq_t[:, 1, :w], attn_t[:, hh2, c0:c1],
                                         attn_t[:, hh2, c0:c1])
                    nc.vector.tensor_add(sq_t[:, 0, :w],
                                         sq_t[:, 0, :w].bitcast(BF16),
                                         sq_t[:, 1, :w])
                pss = psum.tile([1, CH], FP32, tag="psA", bufs=2)
                nc.tensor.matmul(pss[:, :w], ones_t[:], sq_t[:, 0, :w],
                                 start=True, stop=True)
                ssb = sb.tile([1, CH], FP32, tag="ssb", bufs=2)
                nc.scalar.copy(ssb[:, :w], pss[:, :w])
                nc.sync.dma_start(ssq_d[:, t * CH + c0:t * CH + c1],
                                  ssb[:, :w])

            def emit_outproj(gA_t, t, ms):
                for m in ms:
                    ob = sb.tile([P, HIDDEN], BF16, tag="ob", bufs=2)
                    for nt in range(4):
                        pso2 = psum.tile([P, 512], FP32, tag="psA", bufs=2)
                        for h in range(HPC):
                            nc.tensor.matmul(
                                pso2[:],
                                gA_t[:, h, m * P:(m + 1) * P],
                                wo_t[:, h, nt * 512:(nt + 1) * 512],
                                start=(h == 0), stop=(h == HPC - 1))
                        nc.vector.tensor_copy(ob[:, nt * 512:(nt + 1) * 512],
                                              pso2[:])
                    nc.sync.dma_start(
                        pout_d[t * CH + m * P: t * CH + (m + 1) * P, :], ob[:])


            for t in range(NT):
                tsl = slice(t * CH, (t + 1) * CH)

                # ---- projections (streaming x tile, weights resident) ----
                if t == 0 and _rep == 0:
                    xt_h = xh_pre
                    xt8_h = xh8_pre
                else:
                    xt_h = []
                    xt8_h = []
                    if any8:
                        for half in range(2):
                            xt8_h.append(x8_tile(half, tsl, vers=(0,)))
                        if NOS_G > 0:
                            for half in range(2):
                                nc.sync.dma_start(
                                    xt8_h[half][:, 1, :, :],
                                    xt8s_r[:, half * HH:(half + 1) * HH, tsl])
                    for half in range(2):
                        xh = sb.tile([P, HH, CH], BF16, tag="xt", bufs=4)
                        nc.sync.dma_start(
                            xh[:], xt_r[:, half * HH:(half + 1) * HH, tsl])
                        xt_h.append(xh)

                q_t = sb.tile([P, HPC, CH], BF16, tag="q", bufs=2)
                k_t = sb.tile([P, HPC, CH], BF16, tag="k", bufs=2)
                g1_t = sb.tile([P, HPC, CH], BF16, tag="g1", bufs=2)
                v_t = sb.tile([P, 4, IN_PC], BF16, tag="v", bufs=2)

                def emit_proj(w8_t, w_t, n8, dst):
                    nmm_tot = n8 // 2 + (HC - n8)
                    descale = (1.0 / W8SCALE) if n8 > 0 else 1.0
                    for h in range(HPC):
                        hsl = slice(h * P, (h + 1) * P)
                        ps = psum.tile([P, CH], FP32, tag="psA", bufs=2)
                        nmm = 0
                        for i8 in range(0, n8, 2):
                            nc.tensor.matmul(
                                ps[:], w8_t[:, i8:i8 + 2, hsl],
                                xt8_h[i8 // HH][:, 0, i8 % HH:i8 % HH + 2, :],
                                start=(nmm == 0), stop=(nmm == nmm_tot - 1),
                                perf_mode=DR)
                            nmm += 1
                        for hc in range(n8, HC):
                            nc.tensor.matmul(
                                ps[:], w_t[:, hc - n8, hsl],
                                xt_h[hc // HH][:, hc % HH, :],
                                start=(nmm == 0), stop=(nmm == nmm_tot - 1))
                            nmm += 1
                        nc.scalar.activation(dst[:, h, :], ps[:], AF.Silu,
                                             scale=descale)

                def emit_gproj():
                    # one-sided fp8 chunks: stationary slots = (w_hi, w_lo)
                    # e4m3 pair, moving planes = (x8, x8/16); plus bf16 tail
                    descale = (1.0 / W8SCALE) if NOS_G > 0 else 1.0
                    for h in range(HPC):
                        hsl = slice(h * P, (h + 1) * P)
                        ps = psum.tile([P, CH], FP32, tag="psA", bufs=2)
                        for hc in range(NOS_G):
                            nc.tensor.matmul(
                                ps[:], wg8_t[:, hc, :, hsl],
                                xt8_h[hc // HH][:, :, hc % HH, :],
                                start=(hc == 0), stop=(hc == HC - 1),
                                perf_mode=DR)
                        for hc in range(NOS_G, HC):
                            nc.tensor.matmul(
                                ps[:], wg_t[:, hc - NOS_G, hsl],
                                xt_h[hc // HH][:, hc % HH, :],
                                start=(hc == 0), stop=(hc == HC - 1))
                        # tanh(a/2); gate = (1+tanh(a/2))/2, 1/2 is in wo
                        nc.scalar.activation(g1_t[:, h, :], ps[:], AF.Tanh,
                                             scale=0.5 * descale)
                        nc.vector.tensor_scalar_add(
                            g1_t[:, h, :], g1_t[:, h, :].bitcast(BF16), 1.0)

                emit_proj(wq8_t, wq_t, N8_Q, q_t)
                emit_proj(wk8_t, wk_t, N8_K, k_t)
                nmm_tot = N8_V // 2 + (HC - N8_V)
                descale = (1.0 / W8SCALE) if N8_V > 0 else 1.0
                for t2 in range(4):
                    ps = psum.tile([P, IN_PC], FP32, tag="psA", bufs=2)
                    nmm = 0
                    for i8 in range(0, N8_V, 2):
                        nc.tensor.matmul(
                            ps[:],
                            xt8_h[i8 // HH][:, 0, i8 % HH:i8 % HH + 2,
                                            t2 * P:(t2 + 1) * P],
                            wv8_t[:, i8:i8 + 2, :],
                            start=(nmm == 0), stop=(nmm == nmm_tot - 1),
                            perf_mode=DR)
                        nmm += 1
                    for hc in range(N8_V, HC):
                        nc.tensor.matmul(
                            ps[:],
                            xt_h[hc // HH][:, hc % HH, t2 * P:(t2 + 1) * P],
                            wv_t[:, hc - N8_V, :],
                            start=(nmm == 0), stop=(nmm == nmm_tot - 1))
                        nmm += 1
                    nc.scalar.activation(v_t[:, t2, :], ps[:], AF.Silu,
                                         scale=descale)

                # gate proj is only consumed by the NEXT tile's gA multiply
                # (except on the very last tile), so emit it after the
                # attention blocks — its weights/x arrive later in the DMA
                # stream and the attention MMs give them runway.
                last = t == NT - 1 and _rep == repeat - 1
                if last:
                    emit_gproj()

                # ---- previous tile's gate + ssq (DVE chains get a full
                # tile of runway); its out-projection is interleaved into
                # the attention blocks below as PE filler ----
                prev = None
                if pending_out is not None:
                    pa, pg, pt = pending_out
                    gA_prev = sb.tile([P, HPC, CH], BF16, tag="gA", bufs=2)
                    nc.vector.tensor_mul(gA_prev[:], pa[:], pg[:])
                    emit_ssq(pa, pt)
                    prev = (gA_prev, pt)

                attn_t = sb.tile([P, HPC, CH], BF16, tag="attn", bufs=2)
                if last:
                    gA_t = sb.tile([P, HPC, CH], BF16, tag="gA", bufs=2)

                # ---- blocked lightning attention ----
                for b in range(CH // BLOCK):
                    t0 = b * BLOCK
                    qd = sb.tile([P, HPC, BLOCK], BF16, tag="qd", bufs=2)
                    nc.vector.tensor_mul(qd[:], q_t[:, :, t0:t0 + BLOCK],
                                         qdec_t[:])
                    s0s, s1s, kns = [], [], []
                    for h in range(HPC):
                        # scores (transposed): s[j, i] = k_j . q_i
                        ps0 = psum.tile([P, BLOCK], FP32, tag="ps_s", bufs=2)
                        nc.tensor.matmul(ps0[:], k_t[:, h, t0:t0 + P],
                                         q_t[:, h, t0:t0 + BLOCK],
                                         start=True, stop=True)
                        s0 = sb.tile([P, BLOCK], BF16, tag="s0", bufs=3)
                        nc.vector.tensor_mul(s0[:], ps0[:], dmask_t[:, h, :])
                        ps1 = psum.tile([P, P], FP32, tag="ps_s", bufs=2)
                        nc.tensor.matmul(ps1[:], k_t[:, h, t0 + P:t0 + BLOCK],
                                         q_t[:, h, t0 + P:t0 + BLOCK],
                                         start=True, stop=True)
                        s1 = sb.tile([P, P], BF16, tag="s1", bufs=3)
                        nc.vector.tensor_mul(s1[:], ps1[:], dmask_t[:, h, :P])
                        # k natural (for kv update), k-decay folded in
                        kn = []
                        for sub in range(2):
                            pst = psum.tile([P, P], BF16, tag="ps_tr", bufs=2)
                            nc.tensor.transpose(
                                pst[:],
                                k_t[:, h, t0 + sub * P:t0 + (sub + 1) * P],
                                ident_t[:])
                            knt = sb.tile([P, P], BF16, tag=f"kn{sub}", bufs=3)
                            nc.scalar.activation(knt[:], pst[:], AF.Copy,
                                                 scale=kdec_t[:, h, sub, :])
                            kn.append(knt)
                        s0s.append(s0); s1s.append(s1); kns.append(kn)
                    # out-proj chunk of tile t-1: PE filler while this
                    # block's masks (DVE) and kv chain land
                    if prev is not None:
                        emit_outproj(prev[0], prev[1], (2 * b, 2 * b + 1))
                    for h in range(HPC):
                        hsl = slice(h * P, (h + 1) * P)
                        # attention out (transposed): inter + intra
                        pso = psum.tile([P, BLOCK], FP32, tag="ps_o", bufs=2)
                        nc.tensor.matmul(pso[:], kvb_t[:, h, :], qd[:, h, :],
                                         start=True, stop=False)
                        nc.tensor.matmul(pso[:], v_t[:, 2 * b, hsl], s0s[h][:],
                                         start=False, stop=False)
                        nc.tensor.matmul(pso[:, P:BLOCK], v_t[:, 2 * b + 1, hsl],
                                         s1s[h][:], start=False, stop=True,
                                         skip_group_check=True)
                        nc.scalar.copy(attn_t[:, h, t0:t0 + BLOCK], pso[:])
                        # kv update: kv = (bd*I)^T kv + (k*kdec)^T v  (one group)
                        psk = psum.tile([P, P], FP32, tag="ps_s", bufs=2)
                        nc.tensor.matmul(psk[:], bdi_t[:, h, :], kvb_t[:, h, :],
                                         start=True, stop=False)
                        nc.tensor.matmul(psk[:], kns[h][0][:], v_t[:, 2 * b, hsl],
                                         start=False, stop=False)
                        nc.tensor.matmul(psk[:], kns[h][1][:],
                                         v_t[:, 2 * b + 1, hsl],
                                         start=False, stop=True)
                        nc.scalar.copy(kvb_t[:, h, :], psk[:])

                    if last:
                        # stream the final tile's tail per half-block so the
                        # kernel doesn't end with an unoverlapped join
                        csl = slice(t0, t0 + BLOCK)
                        nc.vector.tensor_mul(gA_t[:, :, csl],
                                             attn_t[:, :, csl],
                                             g1_t[:, :, csl])
                        emit_outproj(gA_t, t, (2 * b, 2 * b + 1))
                        emit_ssq(attn_t, t, t0, t0 + BLOCK)

                if not last:
                    emit_gproj()
                    pending_out = (attn_t, g1_t, t)

    nc.compile()
    return nc


_NC_CACHE = {}


def _get_nc(repeat=1, phases="AB", nA=1, nB=1):
    key = (repeat, phases, nA, nB)
    if key not in _NC_CACHE:
        _NC_CACHE[key] = build_nc(repeat, phases, nA, nB)
    return _NC_CACHE[key]


def make_in_maps(inputs):
    np_bf = ml_dtypes.bfloat16
    np_f8 = ml_dtypes.float8_e4m3

    hs = np.ascontiguousarray(np.asarray(inputs["hidden_states"], dtype=np.float32))
    w_qkv = np.asarray(inputs["w_qkv"], dtype=np.float32)
    w_gate = np.asarray(inputs["w_gate"], dtype=np.float32)
    w_out = np.asarray(inputs["w_out"], dtype=np.float32)
    norm_weight = np.asarray(inputs["norm_weight"], dtype=np.float32)
    slope_rate = np.asarray(inputs["slope_rate"], dtype=np.float32).reshape(NUM_HEADS)
    kv_cache = np.asarray(inputs["kv_cache"], dtype=np.float32)

    xt_f32 = np.ascontiguousarray(hs.T)                   # [HIDDEN, SEQ]
    xt = xt_f32.astype(np_bf)
    any8 = max(N8_Q, N8_K, N8_V, NOS_G) > 0
    if any8:
        xt8 = np.clip(xt_f32, -240, 240).astype(np_f8)
    if NOS_G > 0:
        xt8s = (xt8.astype(np.float32) / 16.0).astype(np_f8)
    wq3 = w_qkv.reshape(HIDDEN, NUM_HEADS, 3 * HEAD_DIM)
    ident = np.eye(P, dtype=np_bf)
    ones = np.ones((P, 1), dtype=np_bf)
    idx = np.arange(BLOCK, dtype=np.float64)

    def w_split(m, w, n8):
        """fp8 low-K part (scaled) + bf16 remainder (scaled iff n8>0)."""
        out = {}
        if n8 > 0:
            out[m + "8"] = np.ascontiguousarray(
                np.clip(w[:n8 * P] * W8SCALE, -240, 240)).astype(np_f8)
            if n8 < HC:
                out[m] = np.ascontiguousarray(
                    w[n8 * P:] * W8SCALE).astype(np_bf)
        else:
            out[m] = np.ascontiguousarray(w).astype(np_bf)
        return out

    in_maps = []
    for c in range(N_CORES):
        s = slope_rate[c * HPC:(c + 1) * HPC].astype(np.float64)  # [HPC]
        wq = np.ascontiguousarray(
            wq3[:, c * HPC:(c + 1) * HPC, 0:HEAD_DIM].reshape(HIDDEN, IN_PC))
        wk = np.ascontiguousarray(
            wq3[:, c * HPC:(c + 1) * HPC, HEAD_DIM:2 * HEAD_DIM].reshape(HIDDEN, IN_PC))
        wv = np.ascontiguousarray(
            wq3[:, c * HPC:(c + 1) * HPC, 2 * HEAD_DIM:3 * HEAD_DIM].reshape(HIDDEN, IN_PC))
        wg = np.ascontiguousarray(w_gate[:, c * IN_PC:(c + 1) * IN_PC])
        # fold RMSNorm weight and the gate's 1/2 into w_out rows
        nw = norm_weight[c * IN_PC:(c + 1) * IN_PC].reshape(IN_PC, 1)
        wo = np.ascontiguousarray(
            0.5 * nw * w_out[c * IN_PC:(c + 1) * IN_PC, :]).astype(np_bf)

        jj = idx[:P][:, None]                            # [128,1]
        ii = idx[None, :]                                # [1,256]
        dmask = (np.exp(-s[:, None, None] * (ii - jj)) * (ii >= jj)).astype(np_bf)
        qdec = np.broadcast_to(
            np.exp(-s[:, None] * (idx[None, :] + 1.0))[:, None, :],
            (HPC, P, BLOCK)).astype(np_bf)
        kdec = np.exp(-s[:, None] * (BLOCK - 1.0 - idx[None, :]))  # [HPC, 256]
        kdec = kdec.reshape(HPC, 2, P, 1).astype(np.float32)
        bdi = (np.exp(-s * BLOCK)[:, None, None]
               * np.eye(P, dtype=np.float64)[None]).astype(np_bf)
        kv0 = np.ascontiguousarray(kv_cache[c * HPC:(c + 1) * HPC])

        im = {
            "xt": xt,
            "wo": wo,
            "qdec": np.ascontiguousarray(qdec),
            "dmask": np.ascontiguousarray(dmask),
            "kdec": kdec,
            "bdi": np.ascontiguousarray(bdi),
            "ident": ident, "ones": ones, "kv0": kv0,
        }
        if any8:
            im["xt8"] = xt8
        if NOS_G > 0:
            im["xt8s"] = xt8s
            # hi/lo e4m3 slot-pair pack of the scaled gate weights
            gs = wg[:NOS_G * P] * W8SCALE
            hi = np.clip(gs, -240, 240).astype(np_f8)
            lo = np.clip((gs - hi.astype(np.float32)) * 16.0,
                         -240, 240).astype(np_f8)
            im["wg8"] = np.ascontiguousarray(
                np.stack([hi, lo], axis=1))       # [NOS_G*P, 2, IN_PC]
            if NOS_G < HC:
                im["wg"] = np.ascontiguousarray(
                    wg[NOS_G * P:] * W8SCALE).astype(np_bf)
        else:
            im["wg"] = np.ascontiguousarray(wg).astype(np_bf)
        im.update(w_split("wq", wq, N8_Q))
        im.update(w_split("wk", wk, N8_K))
        im.update(w_split("wv", wv, N8_V))
        in_maps.append(im)
    return in_maps


def combine_outputs(results):
    pout = results[0]["pout"].astype(np.float32)
    ssq = results[0]["ssq"].reshape(SEQ).astype(np.float32).copy()
    for r in results[1:]:
        pout += r["pout"].astype(np.float32)
        ssq += r["ssq"].reshape(SEQ)
    scale = 1.0 / np.sqrt(ssq / INNER + EPS)
    return (pout * scale[:, None]).astype(np.float32)


def kernel(**inputs):
    nc = _get_nc(1)
    in_maps = make_in_maps(inputs)
    res = run_bass_kernel_spmd(nc, in_maps, core_ids=list(range(N_CORES)))
    return combine_outputs(res.results)
